# revision 29
# baseline (speedup 1.0000x reference)
"""Causal self-attention kernel for 8 Trainium2 NeuronCores.

Problem (hardcoded): x [4, 2048, 768] f32, W [768, 2304] f32, b [2304] f32.
reference: qkv = x@W+b; 8 heads, head_dim 96; causal softmax attention.

Sharding: core c handles batch c//2 and heads 4*(c%2) .. 4*(c%2)+3
(data-parallel over batch x tensor-parallel over heads). Host shards
inputs / gathers outputs around one SPMD NEFF; no device collectives.

Per-core device program:
  - projection: qT,kT computed transposed [96, seq] in f32r (bias added as
    per-partition scalar), v computed natural [seq, 96] (bias added via a
    K=1 matmul against a ones row), stored bf16 with a ones column so the
    PV matmul also produces the softmax denominator.
  - attention: S^T[k,q] = kT.T @ qT per 512-wide q block (f32r), block
    pairs packed contiguously in PSUM; exp (scalar engine, no max
    subtraction; logits ~N(0,1)) writes P^T bf16 to SBUF; causal = skip
    upper blocks + affine_select on diagonal 128x128 chunks (gpsimd).
  - PV in natural layout: O[q,d] += pt[:,qchunk].T @ va per k-block,
    accumulated in PSUM [128, 4, 97]; col 96 is the denominator; final
    per-row scale by reciprocal, DMA out. No transposes needed.
"""

import functools
from contextlib import ExitStack

import numpy as np

import concourse.bacc as bacc
import concourse.bass as bass
import concourse.mybir as mybir
import concourse.tile as tile
from concourse.bass_utils import run_bass_kernel_spmd

F32 = mybir.dt.float32
F32R = mybir.dt.float32r
BF16 = mybir.dt.bfloat16

B, N, C, H = 4, 2048, 768, 8
D = C // H            # 96
NCORES = 8
LH = 4                # local heads per core
KC = C // 128         # 6 contraction chunks
NB = N // 512         # 4 seq blocks of 512
OUTC = LH * D         # 384
SCALE = float(1.0 / np.sqrt(np.float32(D)))
UNROLL2 = False   # kept for compat: timer/test drive UNROLL instead
UNROLL = 1        # bodies per loop iteration (reps>1 builds)


@functools.lru_cache(maxsize=8)
def build(reps=1, use_f32r=True, concat=1):
    MDT = F32R if use_f32r else F32
    nc = bacc.Bacc("TRN2", target_bir_lowering=False, debug=False,
                   num_devices=NCORES)
    xt_d = nc.dram_tensor("xt", [C, N], BF16, kind="ExternalInput")
    wqk_d = nc.dram_tensor("wqk", [C, 2 * LH * D], BF16, kind="ExternalInput")
    wv_d = nc.dram_tensor("wv", [C, LH * D], BF16, kind="ExternalInput")
    bqk_d = nc.dram_tensor("bqk", [D, 2 * LH], F32, kind="ExternalInput")
    bv_d = nc.dram_tensor("bv", [1, LH * D], MDT, kind="ExternalInput")
    out_d = nc.dram_tensor("out", [N, OUTC], F32, kind="ExternalOutput")

    xt_v = xt_d.ap().rearrange("(kc p) n -> p kc n", p=128)
    wqk_v = wqk_d.ap().rearrange("(kc p) m -> p kc m", p=128)
    wv_v = wv_d.ap().rearrange("(kc p) m -> p kc m", p=128)
    out_v = out_d.ap().rearrange("(qq t p) c -> qq p t c", t=4, p=128)

    with tile.TileContext(nc) as tc, ExitStack() as ctx:
        const = ctx.enter_context(tc.tile_pool(name="const", bufs=1))
        wpool = ctx.enter_context(tc.tile_pool(name="w", bufs=1))
        xpool = ctx.enter_context(tc.tile_pool(name="x", bufs=3))
        qkpool = ctx.enter_context(tc.tile_pool(name="qk", bufs=NB))
        vpool = ctx.enter_context(tc.tile_pool(name="vaug", bufs=NB))
        ppool = ctx.enter_context(tc.tile_pool(name="p", bufs=12))
        rpool = ctx.enter_context(tc.tile_pool(name="r", bufs=4))
        spool = ctx.enter_context(tc.tile_pool(name="stage", bufs=3))
        ps_proj = ctx.enter_context(
            tc.tile_pool(name="ps_proj", bufs=2, space="PSUM"))
        ps_s = ctx.enter_context(
            tc.tile_pool(name="ps_s", bufs=2, space="PSUM"))
        ps_o = ctx.enter_context(
            tc.tile_pool(name="ps_o", bufs=2, space="PSUM"))

        # one-time constants
        ones = const.tile([1, 128], F32)
        nc.gpsimd.memset(ones[:], 1.0)
        ones_r = const.tile([1, 128], MDT)
        nc.vector.tensor_copy(ones_r[:], ones[:])
        vones = const.tile([128, 4, LH, 1], BF16)
        nc.gpsimd.memset(vones[:], 1.0)

        wqk_sb = wpool.tile([128, KC, 2 * LH * D], BF16, tag="wqk")
        wv_sb = wpool.tile([128, KC, LH * D], BF16, tag="wv")
        # v weights + biases first (v projection starts ASAP);
        # bulkier qk weights go on the scalar engine's DMA queue
        nc.sync.dma_start(wv_sb[:], wv_v[:])
        bqk_sb = wpool.tile([D, 2 * LH], F32, tag="bqk")
        nc.sync.dma_start(bqk_sb[:], bqk_d.ap())
        bv_sb = wpool.tile([1, LH * D], MDT, tag="bv")
        nc.sync.dma_start(bv_sb[:], bv_d.ap())
        for kc in range(KC):
            nc.scalar.dma_start(wqk_sb[:, kc, :], wqk_v[:, kc, :])
        # broadcast v bias to all 128 partitions once (K=1 matmul)
        bvb_sb = wpool.tile([128, LH, D], F32, tag="bvb")
        bvps = ps_proj.tile([128, 512], F32, tag="proj")
        nc.tensor.matmul(bvps[:, 0:LH * D], ones_r[:, :], bv_sb[:, :],
                         start=True, stop=True)
        nc.vector.tensor_copy(
            bvb_sb[:], bvps[:, 0:LH * D].rearrange("p (h d) -> p h d", h=LH))

        def body(scalar_xt0=False):
            qk_tiles = []
            va_tiles = []

            def proj(nb):
                # ---- load x block (transposed layout), one batched DMA ----
                xt_sb = xpool.tile([128, KC, 512], BF16, tag="xt")
                # one-shot only: overlap block-0 x load with wv on sync
                xq = nc.scalar if (nb == 0 and scalar_xt0) else nc.sync
                xq.dma_start(
                    xt_sb[:], xt_v[:, :, nb * 512:(nb + 1) * 512])

                # ---- v projection (natural layout) for this seq block ----
                va = vpool.tile([128, 4, LH, D + 1], BF16, tag="va")
                nc.vector.tensor_copy(va[:, :, :, D:D + 1], vones[:])
                for mt in range(4):
                    vps = ps_proj.tile([128, 512], F32, tag="proj")
                    for kc in range(KC):
                        nc.tensor.matmul(
                            vps[:, 0:LH * D],
                            xt_sb[:, kc, mt * 128:(mt + 1) * 128],
                            wv_sb[:, kc, :],
                            start=(kc == 0), stop=(kc == KC - 1))
                    # bias added during PSUM->SBUF copy
                    nc.vector.tensor_add(
                        va[:, mt, :, 0:D],
                        vps[:, 0:LH * D].rearrange("p (h d) -> p h d", h=LH),
                        bvb_sb[:])
                va_tiles.append(va)

                # ---- q,k projection (transposed layout) for this block ----
                qk_h = []
                for hh in range(LH):
                    qk = qkpool.tile([D, 2, 512], BF16, tag=f"qk{hh}")
                    for t in range(2):
                        m = 2 * hh + t
                        qps = ps_proj.tile([128, 512], F32, tag="proj")
                        for kc in range(KC):
                            nc.tensor.matmul(
                                qps[0:D, :],
                                wqk_sb[:, kc, m * D:(m + 1) * D],
                                xt_sb[:, kc, :],
                                start=(kc == 0), stop=(kc == KC - 1))
                        nc.vector.tensor_scalar_add(
                            qk[:, t, :], qps[0:D, :], bqk_sb[:, m:m + 1])
                    qk_h.append(qk)
                qk_tiles.append(qk_h)

            def attn(Q, heads):
                for h in heads:
                    # S^T + exp for every k-block pair of this head
                    jinfo = {}
                    for pr in range(2 * Q + 2):
                        sps = ps_s.tile([128, 1024], F32, tag="s")
                        pt = ppool.tile([128, 1024], BF16, tag="p")
                        info = []
                        cb = 0
                        for idx in range(2):
                            j = 2 * pr + idx
                            qoff = max(512 * Q, 128 * j)
                            width = 512 * (Q + 1) - qoff
                            info.append((j, qoff, cb))
                            nc.tensor.matmul(
                                sps[:, cb:cb + width],
                                qk_tiles[j // 4][h][
                                    :, 1,
                                    (j % 4) * 128:(j % 4) * 128 + 128],
                                qk_tiles[Q][h][
                                    :, 0,
                                    qoff - 512 * Q:qoff - 512 * Q + width],
                                start=True, stop=True)
                            cb += width
                        nc.scalar.activation(
                            pt[:, 0:cb], sps[:, 0:cb],
                            mybir.ActivationFunctionType.Exp, scale=SCALE)
                        for (j, qoff, c0) in info:
                            if j >= 4 * Q:  # diagonal chunk: causal mask
                                nc.gpsimd.affine_select(
                                    out=pt[:, c0:c0 + 128],
                                    in_=pt[:, c0:c0 + 128],
                                    compare_op=mybir.AluOpType.is_ge,
                                    fill=0.0, base=0, pattern=[[1, 128]],
                                    channel_multiplier=-1)
                            jinfo[j] = (pt, qoff, c0)
                    # PV natural, two waves of two q-chunks; each q-chunk
                    # owns a PSUM bank (one accumulation group per bank)
                    stage = spool.tile([128, 4, D], F32, tag="stage")
                    rr = rpool.tile([128, 4], F32, tag="r")
                    for w in range(2):
                        o_a = ps_o.tile([128, D + 1], F32, tag="o")
                        o_b = ps_o.tile([128, D + 1], F32, tag="o")
                        otile = [o_a, o_b]
                        for j in range(4 * Q + 2 * w + 2):
                            pt, qoff, c0 = jinfo[j]
                            for wi in range(2):
                                qi = 2 * w + wi
                                if j > 4 * Q + qi:
                                    continue
                                loc = 128 * (4 * Q + qi) - qoff
                                nc.tensor.matmul(
                                    otile[wi][:],
                                    pt[:, c0 + loc:c0 + loc + 128],
                                    va_tiles[j // 4][:, j % 4, h, :],
                                    start=(j == 0), stop=(j == 4 * Q + qi))
                        for wi in range(2):
                            qi = 2 * w + wi
                            nc.vector.reciprocal(
                                rr[:, qi:qi + 1], otile[wi][:, D:D + 1])
                            nc.vector.tensor_scalar_mul(
                                stage[:, qi, :], otile[wi][:, 0:D],
                                rr[:, qi:qi + 1])
                    nc.sync.dma_start(
                        out_v[Q, :, :, h * D:(h + 1) * D], stage[:])

            for nb in range(NB):
                proj(nb)
                attn(nb, range(LH))

        if reps == 1:
            for ci in range(concat):
                body(scalar_xt0=(ci == 0))
        else:
            nun = 2 if UNROLL2 else UNROLL
            with tc.For_i(0, reps, 1):
                for _ in range(max(1, nun)):
                    body()

    nc.compile()
    return nc


def f32r_round(a):
    """Round fp32 array to f32r precision (11-bit mantissa, RNE)."""
    u = np.ascontiguousarray(a, dtype=np.float32).view(np.uint32)
    u = (u + 0x7FF + ((u >> 12) & 1)) & np.uint32(0xFFFFF000)
    return u.view(np.float32)


def shard_inputs(x, W, b, use_f32r=True):
    """Full inputs -> per-core in_maps (numpy, fp32)."""
    x = np.asarray(x, dtype=np.float32)
    W = np.asarray(W, dtype=np.float32)
    b = np.asarray(b, dtype=np.float32)
    if use_f32r:
        # round once globally (elementwise, commutes with slicing below)
        x = f32r_round(x)
        W = f32r_round(W)
    import ml_dtypes
    rnd = lambda a: np.ascontiguousarray(a, dtype=np.float32)
    bfc = lambda a: np.ascontiguousarray(a).astype(ml_dtypes.bfloat16)
    in_maps = []
    for c in range(NCORES):
        bc, g = divmod(c, 2)
        h0 = g * LH
        qcols = [W[:, 0 * C + (h0 + h) * D:0 * C + (h0 + h + 1) * D]
                 for h in range(LH)]
        kcols = [W[:, 1 * C + (h0 + h) * D:1 * C + (h0 + h + 1) * D]
                 for h in range(LH)]
        vcols = [W[:, 2 * C + (h0 + h) * D:2 * C + (h0 + h + 1) * D]
                 for h in range(LH)]
        wqk = np.concatenate(
            [m for h in range(LH) for m in (qcols[h], kcols[h])], axis=1)
        wv = np.concatenate(vcols, axis=1)
        bqk = np.stack(
            [b[t * C + (h0 + h) * D:t * C + (h0 + h + 1) * D]
             for h in range(LH) for t in (0, 1)], axis=1)
        bv = np.concatenate(
            [b[2 * C + (h0 + h) * D:2 * C + (h0 + h + 1) * D]
             for h in range(LH)])[None, :]
        in_maps.append({
            "xt": bfc(x[bc].T),
            "wqk": bfc(wqk),
            "wv": bfc(wv),
            "bqk": np.ascontiguousarray(bqk),
            "bv": rnd(bv),
        })
    return in_maps


def gather_outputs(results):
    """Per-core results -> full [B, N, C] output."""
    out = np.empty((B, N, C), dtype=np.float32)
    for c in range(NCORES):
        bc, g = divmod(c, 2)
        out[bc, :, g * OUTC:(g + 1) * OUTC] = results[c]["out"]
    return out


def kernel(x, W, b):
    nc = build(reps=1, use_f32r=True)
    in_maps = shard_inputs(x, W, b, use_f32r=True)
    res = run_bass_kernel_spmd(nc, in_maps, core_ids=list(range(NCORES)))
    return gather_outputs(res.results)


# revision 31
# speedup vs baseline: 1.3396x; 1.3396x over previous
"""Causal self-attention kernel for 8 Trainium2 NeuronCores.

Problem (hardcoded): x [4, 2048, 768] f32, W [768, 2304] f32, b [2304] f32.
reference: qkv = x@W+b; 8 heads, head_dim 96; causal softmax attention.

Sharding: core c handles batch c//2 and heads 4*(c%2) .. 4*(c%2)+3
(data-parallel over batch x tensor-parallel over heads). Host shards
inputs / gathers outputs around one SPMD NEFF; no device collectives.

Per-core device program:
  - projection: qT,kT computed transposed [96, seq] in f32r (bias added as
    per-partition scalar), v computed natural [seq, 96] (bias added via a
    K=1 matmul against a ones row), stored bf16 with a ones column so the
    PV matmul also produces the softmax denominator.
  - attention: S^T[k,q] = kT.T @ qT per 512-wide q block (f32r), block
    pairs packed contiguously in PSUM; exp (scalar engine, no max
    subtraction; logits ~N(0,1)) writes P^T bf16 to SBUF; causal = skip
    upper blocks + affine_select on diagonal 128x128 chunks (gpsimd).
  - PV in natural layout: O[q,d] += pt[:,qchunk].T @ va per k-block,
    accumulated in PSUM [128, 4, 97]; col 96 is the denominator; final
    per-row scale by reciprocal, DMA out. No transposes needed.
"""

import functools
from contextlib import ExitStack

import numpy as np

import concourse.bacc as bacc
import concourse.bass as bass
import concourse.mybir as mybir
import concourse.tile as tile
from concourse.bass_utils import run_bass_kernel_spmd

F32 = mybir.dt.float32
F32R = mybir.dt.float32r
BF16 = mybir.dt.bfloat16

B, N, C, H = 4, 2048, 768, 8
D = C // H            # 96
NCORES = 8
LH = 4                # local heads per core
KC = C // 128         # 6 contraction chunks
NB = N // 512         # 4 seq blocks of 512
OUTC = LH * D         # 384
SCALE = float(1.0 / np.sqrt(np.float32(D)))
UNROLL2 = False   # kept for compat: timer/test drive UNROLL instead
UNROLL = 1        # bodies per loop iteration (reps>1 builds)


@functools.lru_cache(maxsize=8)
def build(reps=1, use_f32r=True, concat=1):
    MDT = F32R if use_f32r else F32
    nc = bacc.Bacc("TRN2", target_bir_lowering=False, debug=False,
                   num_devices=NCORES)
    xt_d = nc.dram_tensor("xt", [C, N], BF16, kind="ExternalInput")
    wqk_d = nc.dram_tensor("wqk", [C, 2 * LH * D], BF16, kind="ExternalInput")
    wv_d = nc.dram_tensor("wv", [C, LH * D], BF16, kind="ExternalInput")
    bqk_d = nc.dram_tensor("bqk", [D, 2 * LH], F32, kind="ExternalInput")
    bv_d = nc.dram_tensor("bv", [1, LH * D], MDT, kind="ExternalInput")
    out_d = nc.dram_tensor("out", [N, OUTC], F32, kind="ExternalOutput")

    xt_v = xt_d.ap().rearrange("(kc p) n -> p kc n", p=128)
    wqk_v = wqk_d.ap().rearrange("(kc p) m -> p kc m", p=128)
    wv_v = wv_d.ap().rearrange("(kc p) m -> p kc m", p=128)
    out_v = out_d.ap().rearrange("(qq t p) c -> qq p t c", t=4, p=128)

    with tile.TileContext(nc) as tc, ExitStack() as ctx:
        const = ctx.enter_context(tc.tile_pool(name="const", bufs=1))
        wpool = ctx.enter_context(tc.tile_pool(name="w", bufs=1))
        xpool = ctx.enter_context(tc.tile_pool(name="x", bufs=3))
        qkpool = ctx.enter_context(tc.tile_pool(name="qk", bufs=NB))
        vpool = ctx.enter_context(tc.tile_pool(name="vaug", bufs=NB))
        ppool = ctx.enter_context(tc.tile_pool(name="p", bufs=12))
        rpool = ctx.enter_context(tc.tile_pool(name="r", bufs=4))
        spool = ctx.enter_context(tc.tile_pool(name="stage", bufs=3))
        ps_proj = ctx.enter_context(
            tc.tile_pool(name="ps_proj", bufs=2, space="PSUM"))
        ps_s = ctx.enter_context(
            tc.tile_pool(name="ps_s", bufs=2, space="PSUM"))
        ps_o = ctx.enter_context(
            tc.tile_pool(name="ps_o", bufs=2, space="PSUM"))

        # one-time constants
        ones = const.tile([1, 128], F32)
        nc.gpsimd.memset(ones[:], 1.0)
        ones_r = const.tile([1, 128], MDT)
        nc.vector.tensor_copy(ones_r[:], ones[:])
        vones = const.tile([128, 4, LH, 1], BF16)
        nc.gpsimd.memset(vones[:], 1.0)

        wqk_sb = wpool.tile([128, KC, 2 * LH * D], BF16, tag="wqk")
        wv_sb = wpool.tile([128, KC, LH * D], BF16, tag="wv")
        # v weights + biases first (v projection starts ASAP);
        # bulkier qk weights go on the scalar engine's DMA queue
        nc.sync.dma_start(wv_sb[:], wv_v[:])
        bqk_sb = wpool.tile([D, 2 * LH], F32, tag="bqk")
        nc.sync.dma_start(bqk_sb[:], bqk_d.ap())
        bv_sb = wpool.tile([1, LH * D], MDT, tag="bv")
        nc.sync.dma_start(bv_sb[:], bv_d.ap())
        # one-shot path: first x block loads ahead of the bulky qk weights
        # on the scalar queue so the v projection can start immediately
        xt0_sb = None
        if reps == 1:
            xt0_sb = xpool.tile([128, KC, 512], BF16, tag="xt")
            nc.scalar.dma_start(xt0_sb[:], xt_v[:, :, 0:512])
        for kc in range(KC):
            nc.scalar.dma_start(wqk_sb[:, kc, :], wqk_v[:, kc, :])
        # broadcast v bias to all 128 partitions once (K=1 matmul)
        bvb_sb = wpool.tile([128, LH, D], F32, tag="bvb")
        bvps = ps_proj.tile([128, 512], F32, tag="proj")
        nc.tensor.matmul(bvps[:, 0:LH * D], ones_r[:, :], bv_sb[:, :],
                         start=True, stop=True)
        nc.vector.tensor_copy(
            bvb_sb[:], bvps[:, 0:LH * D].rearrange("p (h d) -> p h d", h=LH))

        def body(preloaded_xt0=None):
            qk_tiles = []
            va_tiles = []

            def proj(nb):
                # ---- load x block (transposed layout), one batched DMA ----
                if nb == 0 and preloaded_xt0 is not None:
                    xt_sb = preloaded_xt0
                else:
                    xt_sb = xpool.tile([128, KC, 512], BF16, tag="xt")
                    nc.sync.dma_start(
                        xt_sb[:], xt_v[:, :, nb * 512:(nb + 1) * 512])

                # ---- v projection (natural layout) for this seq block ----
                va = vpool.tile([128, 4, LH, D + 1], BF16, tag="va")
                nc.vector.tensor_copy(va[:, :, :, D:D + 1], vones[:])
                for mt in range(4):
                    vps = ps_proj.tile([128, 512], F32, tag="proj")
                    for kc in range(KC):
                        nc.tensor.matmul(
                            vps[:, 0:LH * D],
                            xt_sb[:, kc, mt * 128:(mt + 1) * 128],
                            wv_sb[:, kc, :],
                            start=(kc == 0), stop=(kc == KC - 1))
                    # bias added during PSUM->SBUF copy
                    nc.vector.tensor_add(
                        va[:, mt, :, 0:D],
                        vps[:, 0:LH * D].rearrange("p (h d) -> p h d", h=LH),
                        bvb_sb[:])
                va_tiles.append(va)

                # ---- q,k projection (transposed layout) for this block ----
                qk_h = []
                for hh in range(LH):
                    qk = qkpool.tile([D, 2, 512], BF16, tag=f"qk{hh}")
                    for t in range(2):
                        m = 2 * hh + t
                        qps = ps_proj.tile([128, 512], F32, tag="proj")
                        for kc in range(KC):
                            nc.tensor.matmul(
                                qps[0:D, :],
                                wqk_sb[:, kc, m * D:(m + 1) * D],
                                xt_sb[:, kc, :],
                                start=(kc == 0), stop=(kc == KC - 1))
                        nc.vector.tensor_scalar_add(
                            qk[:, t, :], qps[0:D, :], bqk_sb[:, m:m + 1])
                    qk_h.append(qk)
                qk_tiles.append(qk_h)

            def attn(Q, heads):
                for h in heads:
                    # S^T + exp for every k-block pair of this head
                    jinfo = {}
                    for pr in range(2 * Q + 2):
                        sps = ps_s.tile([128, 1024], F32, tag="s")
                        pt = ppool.tile([128, 1024], BF16, tag="p")
                        info = []
                        cb = 0
                        for idx in range(2):
                            j = 2 * pr + idx
                            qoff = max(512 * Q, 128 * j)
                            width = 512 * (Q + 1) - qoff
                            info.append((j, qoff, cb))
                            nc.tensor.matmul(
                                sps[:, cb:cb + width],
                                qk_tiles[j // 4][h][
                                    :, 1,
                                    (j % 4) * 128:(j % 4) * 128 + 128],
                                qk_tiles[Q][h][
                                    :, 0,
                                    qoff - 512 * Q:qoff - 512 * Q + width],
                                start=True, stop=True)
                            cb += width
                        nc.scalar.activation(
                            pt[:, 0:cb], sps[:, 0:cb],
                            mybir.ActivationFunctionType.Exp, scale=SCALE)
                        for (j, qoff, c0) in info:
                            if j >= 4 * Q:  # diagonal chunk: causal mask
                                nc.gpsimd.affine_select(
                                    out=pt[:, c0:c0 + 128],
                                    in_=pt[:, c0:c0 + 128],
                                    compare_op=mybir.AluOpType.is_ge,
                                    fill=0.0, base=0, pattern=[[1, 128]],
                                    channel_multiplier=-1)
                            jinfo[j] = (pt, qoff, c0)
                    # PV natural, two waves of two q-chunks; each q-chunk
                    # owns a PSUM bank (one accumulation group per bank)
                    stage = spool.tile([128, 4, D], F32, tag="stage")
                    rr = rpool.tile([128, 4], F32, tag="r")
                    for w in range(2):
                        o_a = ps_o.tile([128, D + 1], F32, tag="o")
                        o_b = ps_o.tile([128, D + 1], F32, tag="o")
                        otile = [o_a, o_b]
                        for j in range(4 * Q + 2 * w + 2):
                            pt, qoff, c0 = jinfo[j]
                            for wi in range(2):
                                qi = 2 * w + wi
                                if j > 4 * Q + qi:
                                    continue
                                loc = 128 * (4 * Q + qi) - qoff
                                nc.tensor.matmul(
                                    otile[wi][:],
                                    pt[:, c0 + loc:c0 + loc + 128],
                                    va_tiles[j // 4][:, j % 4, h, :],
                                    start=(j == 0), stop=(j == 4 * Q + qi))
                        for wi in range(2):
                            qi = 2 * w + wi
                            nc.vector.reciprocal(
                                rr[:, qi:qi + 1], otile[wi][:, D:D + 1])
                            nc.vector.tensor_scalar_mul(
                                stage[:, qi, :], otile[wi][:, 0:D],
                                rr[:, qi:qi + 1])
                    nc.sync.dma_start(
                        out_v[Q, :, :, h * D:(h + 1) * D], stage[:])

            for nb in range(NB):
                proj(nb)
                attn(nb, range(LH))

        if reps == 1:
            for ci in range(concat):
                body(preloaded_xt0=(xt0_sb if ci == 0 else None))
        else:
            nun = 2 if UNROLL2 else UNROLL
            with tc.For_i(0, reps, 1):
                for _ in range(max(1, nun)):
                    body()

    nc.compile()
    return nc


def f32r_round(a):
    """Round fp32 array to f32r precision (11-bit mantissa, RNE)."""
    u = np.ascontiguousarray(a, dtype=np.float32).view(np.uint32)
    u = (u + 0x7FF + ((u >> 12) & 1)) & np.uint32(0xFFFFF000)
    return u.view(np.float32)


def shard_inputs(x, W, b, use_f32r=True):
    """Full inputs -> per-core in_maps (numpy, fp32)."""
    x = np.asarray(x, dtype=np.float32)
    W = np.asarray(W, dtype=np.float32)
    b = np.asarray(b, dtype=np.float32)
    if use_f32r:
        # round once globally (elementwise, commutes with slicing below)
        x = f32r_round(x)
        W = f32r_round(W)
    import ml_dtypes
    rnd = lambda a: np.ascontiguousarray(a, dtype=np.float32)
    bfc = lambda a: np.ascontiguousarray(a).astype(ml_dtypes.bfloat16)
    in_maps = []
    for c in range(NCORES):
        bc, g = divmod(c, 2)
        h0 = g * LH
        qcols = [W[:, 0 * C + (h0 + h) * D:0 * C + (h0 + h + 1) * D]
                 for h in range(LH)]
        kcols = [W[:, 1 * C + (h0 + h) * D:1 * C + (h0 + h + 1) * D]
                 for h in range(LH)]
        vcols = [W[:, 2 * C + (h0 + h) * D:2 * C + (h0 + h + 1) * D]
                 for h in range(LH)]
        wqk = np.concatenate(
            [m for h in range(LH) for m in (qcols[h], kcols[h])], axis=1)
        wv = np.concatenate(vcols, axis=1)
        bqk = np.stack(
            [b[t * C + (h0 + h) * D:t * C + (h0 + h + 1) * D]
             for h in range(LH) for t in (0, 1)], axis=1)
        bv = np.concatenate(
            [b[2 * C + (h0 + h) * D:2 * C + (h0 + h + 1) * D]
             for h in range(LH)])[None, :]
        in_maps.append({
            "xt": bfc(x[bc].T),
            "wqk": bfc(wqk),
            "wv": bfc(wv),
            "bqk": np.ascontiguousarray(bqk),
            "bv": rnd(bv),
        })
    return in_maps


def gather_outputs(results):
    """Per-core results -> full [B, N, C] output."""
    out = np.empty((B, N, C), dtype=np.float32)
    for c in range(NCORES):
        bc, g = divmod(c, 2)
        out[bc, :, g * OUTC:(g + 1) * OUTC] = results[c]["out"]
    return out


def kernel(x, W, b):
    nc = build(reps=1, use_f32r=True)
    in_maps = shard_inputs(x, W, b, use_f32r=True)
    res = run_bass_kernel_spmd(nc, in_maps, core_ids=list(range(NCORES)))
    return gather_outputs(res.results)
